# Initial kernel scaffold
#
"""Bass/Trainium2 kernel for BiDirectionalCrossAttention (8-core SPMD).

Sharding: 8 cores = 4 batches x 2 head-groups (4 heads each).
Each core computes, for its (batch b, head-group g):
  - Q/K projections restricted to its 256 channels, channel-major [chan, token]
  - V projection in [token, chan] layout with interleaved ones-columns
    (softmax denominator falls out of the attn@V matmul for free)
  - scoresT[kv, q] per head, exp on ScalarE, attn@V accumulation on PE
  - partial output projection Wout[:, cols_g] @ out_g  -> [512, 1024]
Host sums the two partials per batch and adds the folded bias
bout' = bout + Wout @ bv (V-bias commutes through softmax since rows sum to 1).
"""

import sys
import os

for _p in ("/opt/trn_rl_repo", "/root/.axon_site/_ro/trn_rl_repo"):
    if os.path.isdir(_p) and _p not in sys.path:
        sys.path.append(_p)

import numpy as np
import ml_dtypes

import concourse.bass as bass
import concourse.mybir as mybir
import concourse.tile as tile
from concourse.bass_utils import run_bass_kernel_spmd
from concourse.tile import ScopedClock

BF16 = mybir.dt.bfloat16
F32 = mybir.dt.float32
F32R = mybir.dt.float32r
NP_BF16 = ml_dtypes.bfloat16

AF = mybir.ActivationFunctionType


def _split_multi_waits(nc: bass.Bass) -> None:
    """The walrus build here allows only one sync-wait per instruction.
    Tile attaches several; hoist the extras onto same-engine NOPs placed
    immediately before the instruction (same per-engine program order)."""
    uid = 0
    for f in nc.m.functions:
        for bb in f.blocks:
            insts = bb.instructions
            out = []
            changed = False
            for inst in insts:
                si = inst.sync_info
                if si is not None and si.on_wait is not None and len(si.on_wait) > 1:
                    waits = list(si.on_wait)
                    for w in waits[:-1]:
                        nop = mybir.InstNoOp(
                            name=f"splitwait-{uid}",
                            engine=inst.engine,
                            ins=[],
                            outs=[],
                            sync_info=mybir.SyncInfo(on_wait=[w], on_update=[]),
                        )
                        uid += 1
                        out.append(nop)
                    inst.sync_info = mybir.SyncInfo(
                        on_wait=[waits[-1]], on_update=list(si.on_update or [])
                    )
                    changed = True
                out.append(inst)
            if changed:
                bb.instructions = out


def _build_program() -> bass.Bass:
    nc = bass.Bass()

    qx_d = nc.declare_dram_parameter("qx", [512, 1024], BF16, isOutput=False)
    kvx_d = nc.declare_dram_parameter("kvx", [512, 2048], BF16, isOutput=False)
    wq_d = nc.declare_dram_parameter("wq", [512, 256], BF16, isOutput=False)
    wk_d = nc.declare_dram_parameter("wk", [512, 256], BF16, isOutput=False)
    wv_d = nc.declare_dram_parameter("wv", [513, 260], BF16, isOutput=False)
    wo_d = nc.declare_dram_parameter("wo", [256, 512], BF16, isOutput=False)
    bq_d = nc.declare_dram_parameter("bq", [128, 2], F32, isOutput=False)
    bk_d = nc.declare_dram_parameter("bk", [128, 2], F32, isOutput=False)
    out_d = nc.declare_dram_parameter("out", [512, 1024], F32, isOutput=True)

    from contextlib import ExitStack

    with tile.TileContext(nc) as tc, ExitStack() as ctx:
        sb = ctx.enter_context(tc.tile_pool(name="sb", bufs=1))
        esb = ctx.enter_context(tc.tile_pool(name="esb", bufs=10))
        small = ctx.enter_context(tc.tile_pool(name="small", bufs=4))
        # PSUM budget (8 banks): "sc" 2 slots x [128,2,512] (2 banks) = 4,
        # "o" 4 slots x 1 bank = 4. Q/K-proj + out-proj borrow "o", V-proj "sc".
        sc_ps = ctx.enter_context(tc.tile_pool(name="scps", bufs=2, space="PSUM"))
        dpool = ctx.enter_context(tc.tile_pool(name="dram", bufs=2, space="DRAM"))
        o_ps = ctx.enter_context(tc.tile_pool(name="ops", bufs=4, space="PSUM"))

        # ---------------- SBUF tiles ----------------
        qx_s = sb.tile([128, 4, 1024], BF16, name="qx", tag="qx")
        kvx_s = sb.tile([128, 4, 2048], BF16, name="kvx", tag="kvx")
        wq_s = sb.tile([128, 4, 256], BF16, name="wq", tag="wq")
        wk_s = sb.tile([128, 4, 256], BF16, name="wk", tag="wk")
        wv_s = sb.tile([128, 4, 260], BF16, name="wv", tag="wv")
        wv_ones = sb.tile([1, 260], BF16, name="wv_ones", tag="wv_ones")
        wo_s = sb.tile([128, 2, 512], BF16, name="wo", tag="wo")
        bq_s = sb.tile([128, 2], F32, name="bq", tag="bq")
        bk_s = sb.tile([128, 2], F32, name="bk", tag="bk")
        ones_row = sb.tile([1, 2048], BF16, name="ones_row", tag="ones_row")
        qt_s = [sb.tile([128, 1024], BF16, name=f"qt{m}", tag=f"qt{m}") for m in range(2)]
        kt_s = [sb.tile([128, 2048], BF16, name=f"kt{m}", tag=f"kt{m}") for m in range(2)]
        v_s = sb.tile([128, 16, 260], BF16, name="v", tag="v")
        ot_s = [sb.tile([128, 1024], BF16, name=f"ot{m}", tag=f"ot{m}") for m in range(2)]

        nc.vector.memset(ones_row[:], 1.0)

        # ---------------- DMAs, consumption order, 3D APs ----------------
        def chunked(d, parts=128):
            return d.rearrange("(k p) n -> p k n", p=parts)

        nc.sync.dma_start(out=wq_s[:], in_=chunked(wq_d))
        nc.sync.dma_start(out=bq_s[:], in_=bq_d[:])
        nc.sync.dma_start(out=qx_s[:, :, 0:512], in_=chunked(qx_d[:, 0:512]))
        nc.sync.dma_start(out=wk_s[:], in_=chunked(wk_d))
        nc.sync.dma_start(out=bk_s[:], in_=bk_d[:])
        nc.sync.dma_start(out=kvx_s[:, :, 0:512], in_=chunked(kvx_d[:, 0:512]))
        nc.sync.dma_start(out=qx_s[:, :, 512:1024], in_=chunked(qx_d[:, 512:1024]))
        nc.sync.dma_start(out=wv_s[:], in_=chunked(wv_d[0:512, :]))
        nc.sync.dma_start(out=wv_ones[:], in_=wv_d[512:513, :])
        for t in range(1, 4):
            nc.sync.dma_start(out=kvx_s[:, :, t * 512:(t + 1) * 512],
                              in_=chunked(kvx_d[:, t * 512:(t + 1) * 512]))
        nc.sync.dma_start(out=wo_s[:], in_=wo_d.rearrange("(m p) n -> p m n", p=128))

        # ---------------- building blocks ----------------
        def qproj_group(m, t):
            ps = o_ps.tile([128, 512], F32, name="o", tag="o", bufs=4)
            for k in range(4):
                nc.tensor.matmul(
                    ps,
                    lhsT=wq_s[:, k, m * 128:(m + 1) * 128],
                    rhs=qx_s[:, k, t * 512:(t + 1) * 512],
                    start=(k == 0), stop=(k == 3),
                )
            nc.vector.tensor_scalar_add(
                out=qt_s[m][:, t * 512:(t + 1) * 512], in0=ps,
                scalar1=bq_s[:, m:m + 1],
            )

        def kproj_group(m, t):
            ps = o_ps.tile([128, 512], F32, name="o", tag="o", bufs=4)
            for k in range(4):
                nc.tensor.matmul(
                    ps,
                    lhsT=wk_s[:, k, m * 128:(m + 1) * 128],
                    rhs=kvx_s[:, k, t * 512:(t + 1) * 512],
                    start=(k == 0), stop=(k == 3),
                )
            nc.vector.tensor_scalar_add(
                out=kt_s[m][:, t * 512:(t + 1) * 512], in0=ps,
                scalar1=bk_s[:, m:m + 1],
            )

        def vproj_tile(tt):
            # [token,260]: cols 65j..65j+63 head-j dims, col 65j+64 == 1.0
            # (ones-row matmul; wv rows 0..511 are zero in those columns)
            ps = sc_ps.tile([128, 260], F32, name="sc", tag="sc")
            for k in range(4):
                nc.tensor.matmul(
                    ps,
                    lhsT=kvx_s[:, k, tt * 128:(tt + 1) * 128],
                    rhs=wv_s[:, k, :],
                    start=(k == 0), stop=False,
                )
            nc.tensor.matmul(
                ps,
                lhsT=ones_row[:, tt * 128:(tt + 1) * 128],
                rhs=wv_ones[:],
                start=False, stop=True,
            )
            nc.vector.tensor_copy(out=v_s[:, tt, :], in_=ps)

        o_tiles = {}

        recips = {}

        def norm_recip(m, t):
            oA, oB = o_tiles[(m, t)]
            # both heads' softmax sums -> one [33,512] reciprocal (DVE time
            # scales with free size; rows 1..31 are don't-care garbage)
            ssb = small.tile([33, 512], F32, name="ssb", tag="ssb")
            nc.vector.tensor_copy(out=ssb[0:1, :], in_=oA[64:65, :])
            nc.vector.tensor_copy(out=ssb[32:33, :], in_=oB[64:65, :])
            recip = small.tile([33, 512], F32, name="recip", tag="recip")
            nc.vector.reciprocal(out=recip, in_=ssb)
            # bounce 1/s through DRAM so it can be re-read with a 0-stride
            # (partition-broadcast) source AP — keeps the PE stream untouched
            sd = dpool.tile([2, 512], F32, name="sd", tag="sd")
            nc.sync.dma_start(out=sd[0:1, :], in_=recip[0:1, :])
            nc.sync.dma_start(out=sd[1:2, :], in_=recip[32:33, :])
            recips[(m, t)] = sd

        def norm_apply(m, t):
            qsl = slice(t * 512, (t + 1) * 512)
            oA, oB = o_tiles.pop((m, t))
            sd = recips.pop((m, t))
            for o_t, row, base in ((oA, 0, 0), (oB, 1, 64)):
                row_ap = sd[row:row + 1, :]
                bsrc = bass.AP(tensor=row_ap.tensor, offset=row_ap.offset,
                               ap=[[0, 64], [1, 512]])
                bcs = small.tile([64, 512], F32, name="bcs", tag="bcs")
                nc.sync.dma_start(out=bcs, in_=bsrc)
                nc.vector.tensor_mul(ot_s[m][base:base + 64, qsl], o_t[0:64, :], bcs)

        fo_tiles = {}

        def outproj_group(t2, mo, engine="vector"):
            if t2 not in fo_tiles:
                fo_tiles[t2] = small.tile([128, 4, 512], F32, name="fo",
                                          tag="fo", bufs=2)
            fo = fo_tiles[t2]
            ps = o_ps.tile([128, 512], F32, name="o", tag="o", bufs=4)
            for m in range(2):
                nc.tensor.matmul(
                    ps,
                    lhsT=wo_s[:, m, mo * 128:(mo + 1) * 128],
                    rhs=ot_s[m][:, t2 * 512:(t2 + 1) * 512],
                    start=(m == 0), stop=(m == 1),
                )
            if engine == "vector":
                nc.vector.tensor_copy(out=fo[:, mo, :], in_=ps)
            else:
                nc.scalar.activation(out=fo[:, mo, :], in_=ps, func=AF.Copy)
            nc.sync.dma_start(
                out=out_d[mo * 128:(mo + 1) * 128, t2 * 512:(t2 + 1) * 512],
                in_=fo[:, mo, :],
            )

        # ---------------- pipelined schedule ----------------
        # 64 global iterations (4 units x 16 kv tiles); scores emitted one
        # iteration ahead so ScalarE's exp stream never waits on PE.
        units = [(0, 0), (1, 0), (0, 1), (1, 1)]
        iters = [(u, i) for u in units for i in range(16)]

        # interleave remaining projections + V tiles + norms + out-proj
        # into the per-iteration PE slack (ACT exp is the steady-state pacer)
        extra = {g: [] for g in range(64)}
        kplan = [(0, 1), (0, 2), (0, 3), (1, 0), (1, 1), (1, 2), (1, 3)]
        for idx, (m_, t_) in enumerate(kplan):
            extra[2 * idx + 1].append(lambda m_=m_, t_=t_: kproj_group(m_, t_))
        extra[0].append(lambda: qproj_group(1, 0))
        for tt in range(16):
            extra[tt].append(lambda tt=tt: vproj_tile(tt))
        post = {
            17: [lambda: norm_recip(0, 0)],
            22: [lambda: norm_apply(0, 0)],
            23: [lambda: qproj_group(0, 1)],
            25: [lambda: qproj_group(1, 1)],
            33: [lambda: norm_recip(1, 0)],
            38: [lambda: norm_apply(1, 0)],
            41: [lambda: outproj_group(0, 0)],
            43: [lambda: outproj_group(0, 1)],
            45: [lambda: outproj_group(0, 2)],
            47: [lambda: outproj_group(0, 3)],
            49: [lambda: norm_recip(0, 1)],
            54: [lambda: norm_apply(0, 1)],
        }

        qproj_group(0, 0)
        kproj_group(0, 0)

        sc_tiles = {}

        def emit_scores(g):
            (m, t), i = iters[g]
            ksl = slice(i * 128, (i + 1) * 128)
            qsl = slice(t * 512, (t + 1) * 512)
            sc = sc_ps.tile([128, 2, 512], F32, name="sc", tag="sc")
            nc.tensor.matmul(
                sc[:, 0, :], lhsT=kt_s[m][0:64, ksl], rhs=qt_s[m][0:64, qsl],
                start=True, stop=True, tile_position=(0, 0),
            )
            nc.tensor.matmul(
                sc[:, 1, :], lhsT=kt_s[m][64:128, ksl], rhs=qt_s[m][64:128, qsl],
                start=True, stop=True, tile_position=(64, 0),
            )
            sc_tiles[g] = sc

        emit_scores(0)
        for g in range(64):
            (m, t), i = iters[g]
            if g + 1 < 64:
                emit_scores(g + 1)
            sc = sc_tiles.pop(g)
            e = esb.tile([128, 2, 512], BF16, name="e", tag="e")
            nc.scalar.activation(out=e[:], in_=sc[:], func=AF.Exp, scale=0.125)
            for fn in extra.get(g, ()):
                fn()
            if i == 0:
                oA = o_ps.tile([65, 512], F32, name="o", tag="o", bufs=4)
                oB = o_ps.tile([65, 512], F32, name="o", tag="o", bufs=4)
                o_tiles[(m, t)] = (oA, oB)
            oA, oB = o_tiles[(m, t)]
            jA, jB = 2 * m, 2 * m + 1
            nc.tensor.matmul(
                oA, lhsT=v_s[:, i, 65 * jA:65 * jA + 65], rhs=e[:, 0, :],
                start=(i == 0), stop=(i == 15),
            )
            nc.tensor.matmul(
                oB, lhsT=v_s[:, i, 65 * jB:65 * jB + 65], rhs=e[:, 1, :],
                start=(i == 0), stop=(i == 15),
            )
            for fn in post.get(g, ()):
                fn()

        norm_recip(1, 1)
        norm_apply(1, 1)
        for mo in range(4):
            outproj_group(1, mo, engine="scalar")

    _split_multi_waits(nc)
    return nc


_PROGRAM = None


def _get_program() -> bass.Bass:
    global _PROGRAM
    if _PROGRAM is None:
        _PROGRAM = _build_program()
    return _PROGRAM


def _prep_core_inputs(c, q, kv, Wqkv, bqkv, Wout):
    b, g = c // 2, c % 2
    cs = slice(256 * g, 256 * g + 256)
    wv_base = Wqkv[1024 + 256 * g:1024 + 256 * g + 256, :].T  # [512, 256]
    wv = np.zeros((513, 260), np.float32)
    for j in range(4):
        wv[0:512, 65 * j:65 * j + 64] = wv_base[:, 64 * j:64 * j + 64]
        wv[512, 65 * j + 64] = 1.0
    return {
        "qx": np.ascontiguousarray(q[b].reshape(512, 1024)).astype(NP_BF16),
        "kvx": np.ascontiguousarray(kv[b].reshape(512, 2048)).astype(NP_BF16),
        "wq": np.ascontiguousarray(Wqkv[cs, :].T).astype(NP_BF16),
        "wk": np.ascontiguousarray(Wqkv[512 + 256 * g:512 + 256 * g + 256, :].T).astype(NP_BF16),
        "wv": wv.astype(NP_BF16),
        "wo": np.ascontiguousarray(Wout[:, cs].T).astype(NP_BF16),
        "bq": np.ascontiguousarray(bqkv[cs].reshape(2, 128).T).astype(np.float32),
        "bk": np.ascontiguousarray(bqkv[512 + 256 * g:512 + 256 * g + 256].reshape(2, 128).T).astype(np.float32),
    }


def kernel(q, kv, Wqkv, bqkv, Wout, bout):
    q = np.asarray(q, np.float32)
    kv = np.asarray(kv, np.float32)
    Wqkv = np.asarray(Wqkv, np.float32)
    bqkv = np.asarray(bqkv, np.float32)
    Wout = np.asarray(Wout, np.float32)
    bout = np.asarray(bout, np.float32)

    nc = _get_program()
    in_maps = [_prep_core_inputs(c, q, kv, Wqkv, bqkv, Wout) for c in range(8)]
    res = run_bass_kernel_spmd(nc, in_maps, list(range(8))).results

    # V-bias folds through softmax (rows sum to 1): bout' = bout + Wout @ bv
    bout_adj = bout + Wout @ bqkv[1024:1536]
    out = np.empty((4, 512, 32, 32), np.float32)
    for b in range(4):
        o = res[2 * b]["out"] + res[2 * b + 1]["out"] + bout_adj[:, None]
        out[b] = o.reshape(512, 32, 32)
    return out



# revision 1
# speedup vs baseline: 1.5070x; 1.5070x over previous
"""Bass/Trainium2 kernel for BiDirectionalCrossAttention (8-core SPMD).

Sharding: 8 cores = 4 batches x 2 head-groups (4 heads each).
Each core computes, for its (batch b, head-group g):
  - Q/K projections restricted to its 256 channels, channel-major [chan, token]
  - V projection in [token, chan] layout with interleaved ones-columns
    (softmax denominator falls out of the attn@V matmul for free)
  - scoresT[kv, q] per head, exp on ScalarE, attn@V accumulation on PE
  - partial output projection Wout[:, cols_g] @ out_g  -> [512, 1024]
Host sums the two partials per batch and adds the folded bias
bout' = bout + Wout @ bv (V-bias commutes through softmax since rows sum to 1).
"""

import sys
import os

for _p in ("/opt/trn_rl_repo", "/root/.axon_site/_ro/trn_rl_repo"):
    if os.path.isdir(_p) and _p not in sys.path:
        sys.path.append(_p)

import numpy as np
import ml_dtypes

import concourse.bass as bass
import concourse.mybir as mybir
import concourse.tile as tile
from concourse.bass_utils import run_bass_kernel_spmd
from concourse.tile import ScopedClock

BF16 = mybir.dt.bfloat16
F32 = mybir.dt.float32
F32R = mybir.dt.float32r
NP_BF16 = ml_dtypes.bfloat16

AF = mybir.ActivationFunctionType


def _split_multi_waits(nc: bass.Bass) -> None:
    """The walrus build here allows only one sync-wait per instruction.
    Tile attaches several; hoist the extras onto same-engine NOPs placed
    immediately before the instruction (same per-engine program order)."""
    uid = 0
    for f in nc.m.functions:
        for bb in f.blocks:
            insts = bb.instructions
            out = []
            changed = False
            for inst in insts:
                si = inst.sync_info
                if si is not None and si.on_wait is not None and len(si.on_wait) > 1:
                    waits = list(si.on_wait)
                    for w in waits[:-1]:
                        nop = mybir.InstNoOp(
                            name=f"splitwait-{uid}",
                            engine=inst.engine,
                            ins=[],
                            outs=[],
                            sync_info=mybir.SyncInfo(on_wait=[w], on_update=[]),
                        )
                        uid += 1
                        out.append(nop)
                    inst.sync_info = mybir.SyncInfo(
                        on_wait=[waits[-1]], on_update=list(si.on_update or [])
                    )
                    changed = True
                out.append(inst)
            if changed:
                bb.instructions = out


def _build_program() -> bass.Bass:
    nc = bass.Bass()

    qx_d = nc.declare_dram_parameter("qx", [512, 1024], BF16, isOutput=False)
    kvx_d = nc.declare_dram_parameter("kvx", [512, 2048], BF16, isOutput=False)
    wq_d = nc.declare_dram_parameter("wq", [512, 256], BF16, isOutput=False)
    wk_d = nc.declare_dram_parameter("wk", [512, 256], BF16, isOutput=False)
    wv_d = nc.declare_dram_parameter("wv", [513, 260], BF16, isOutput=False)
    wo_d = nc.declare_dram_parameter("wo", [256, 512], BF16, isOutput=False)
    bq_d = nc.declare_dram_parameter("bq", [128, 2], F32, isOutput=False)
    bk_d = nc.declare_dram_parameter("bk", [128, 2], F32, isOutput=False)
    out_d = nc.declare_dram_parameter("out", [512, 1024], F32, isOutput=True)

    from contextlib import ExitStack

    with tile.TileContext(nc) as tc, ExitStack() as ctx:
        sb = ctx.enter_context(tc.tile_pool(name="sb", bufs=1))
        esb = ctx.enter_context(tc.tile_pool(name="esb", bufs=10))
        small = ctx.enter_context(tc.tile_pool(name="small", bufs=4))
        # PSUM budget (8 banks): "sc" 2 slots x [128,2,512] (2 banks) = 4,
        # "o" 4 slots x 1 bank = 4. Q/K-proj + out-proj borrow "o", V-proj "sc".
        sc_ps = ctx.enter_context(tc.tile_pool(name="scps", bufs=2, space="PSUM"))
        dpool = ctx.enter_context(tc.tile_pool(name="dram", bufs=2, space="DRAM"))
        o_ps = ctx.enter_context(tc.tile_pool(name="ops", bufs=4, space="PSUM"))

        # ---------------- SBUF tiles ----------------
        qx_s = sb.tile([128, 4, 1024], BF16, name="qx", tag="qx")
        kvx_s = sb.tile([128, 4, 2048], BF16, name="kvx", tag="kvx")
        wq_s = sb.tile([128, 4, 256], BF16, name="wq", tag="wq")
        wk_s = sb.tile([128, 4, 256], BF16, name="wk", tag="wk")
        wv_s = sb.tile([128, 4, 260], BF16, name="wv", tag="wv")
        wv_ones = sb.tile([1, 260], BF16, name="wv_ones", tag="wv_ones")
        wo_s = sb.tile([128, 2, 512], BF16, name="wo", tag="wo")
        bq_s = sb.tile([128, 2], F32, name="bq", tag="bq")
        bk_s = sb.tile([128, 2], F32, name="bk", tag="bk")
        ones_row = sb.tile([1, 2048], BF16, name="ones_row", tag="ones_row")
        qt_s = [sb.tile([128, 1024], BF16, name=f"qt{m}", tag=f"qt{m}") for m in range(2)]
        kt_s = [sb.tile([128, 2048], BF16, name=f"kt{m}", tag=f"kt{m}") for m in range(2)]
        v_s = sb.tile([128, 16, 260], BF16, name="v", tag="v")
        ot_s = [sb.tile([128, 1024], BF16, name=f"ot{m}", tag=f"ot{m}") for m in range(2)]

        nc.vector.memset(ones_row[:], 1.0)

        # ---------------- DMAs, consumption order, 3D APs ----------------
        def chunked(d, parts=128):
            return d.rearrange("(k p) n -> p k n", p=parts)

        nc.sync.dma_start(out=wq_s[:], in_=chunked(wq_d))
        nc.sync.dma_start(out=bq_s[:], in_=bq_d[:])
        nc.sync.dma_start(out=qx_s[:, :, 0:512], in_=chunked(qx_d[:, 0:512]))
        nc.sync.dma_start(out=wk_s[:], in_=chunked(wk_d))
        nc.sync.dma_start(out=bk_s[:], in_=bk_d[:])
        nc.sync.dma_start(out=kvx_s[:, :, 0:512], in_=chunked(kvx_d[:, 0:512]))
        nc.sync.dma_start(out=qx_s[:, :, 512:1024], in_=chunked(qx_d[:, 512:1024]))
        nc.sync.dma_start(out=wv_s[:], in_=chunked(wv_d[0:512, :]))
        nc.sync.dma_start(out=wv_ones[:], in_=wv_d[512:513, :])
        for t in range(1, 4):
            nc.sync.dma_start(out=kvx_s[:, :, t * 512:(t + 1) * 512],
                              in_=chunked(kvx_d[:, t * 512:(t + 1) * 512]))
        nc.sync.dma_start(out=wo_s[:], in_=wo_d.rearrange("(m p) n -> p m n", p=128))

        # ---------------- building blocks ----------------
        def qproj_group(m, t):
            ps = o_ps.tile([128, 512], F32, name="o", tag="o", bufs=4)
            for k in range(4):
                nc.tensor.matmul(
                    ps,
                    lhsT=wq_s[:, k, m * 128:(m + 1) * 128],
                    rhs=qx_s[:, k, t * 512:(t + 1) * 512],
                    start=(k == 0), stop=(k == 3),
                )
            nc.vector.tensor_scalar_add(
                out=qt_s[m][:, t * 512:(t + 1) * 512], in0=ps,
                scalar1=bq_s[:, m:m + 1],
            )

        def kproj_group(m, t):
            ps = o_ps.tile([128, 512], F32, name="o", tag="o", bufs=4)
            for k in range(4):
                nc.tensor.matmul(
                    ps,
                    lhsT=wk_s[:, k, m * 128:(m + 1) * 128],
                    rhs=kvx_s[:, k, t * 512:(t + 1) * 512],
                    start=(k == 0), stop=(k == 3),
                )
            nc.vector.tensor_scalar_add(
                out=kt_s[m][:, t * 512:(t + 1) * 512], in0=ps,
                scalar1=bk_s[:, m:m + 1],
            )

        def vproj_tile(tt):
            # [token,260]: cols 65j..65j+63 head-j dims, col 65j+64 == 1.0
            # (ones-row matmul; wv rows 0..511 are zero in those columns)
            ps = sc_ps.tile([128, 260], F32, name="sc", tag="sc")
            for k in range(4):
                nc.tensor.matmul(
                    ps,
                    lhsT=kvx_s[:, k, tt * 128:(tt + 1) * 128],
                    rhs=wv_s[:, k, :],
                    start=(k == 0), stop=False,
                )
            nc.tensor.matmul(
                ps,
                lhsT=ones_row[:, tt * 128:(tt + 1) * 128],
                rhs=wv_ones[:],
                start=False, stop=True,
            )
            nc.vector.tensor_copy(out=v_s[:, tt, :], in_=ps)

        o_tiles = {}

        recips = {}

        def norm_recip(m, t):
            oA, oB = o_tiles[(m, t)]
            # both heads' softmax sums -> one [33,512] reciprocal (DVE time
            # scales with free size; rows 1..31 are don't-care garbage)
            ssb = small.tile([33, 512], F32, name="ssb", tag="ssb")
            nc.vector.tensor_copy(out=ssb[0:1, :], in_=oA[64:65, :])
            nc.vector.tensor_copy(out=ssb[32:33, :], in_=oB[64:65, :])
            recip = small.tile([33, 512], F32, name="recip", tag="recip")
            nc.vector.reciprocal(out=recip, in_=ssb)
            # bounce 1/s through DRAM so it can be re-read with a 0-stride
            # (partition-broadcast) source AP — keeps the PE stream untouched
            sd = dpool.tile([2, 512], F32, name="sd", tag="sd")
            nc.sync.dma_start(out=sd[0:1, :], in_=recip[0:1, :])
            nc.sync.dma_start(out=sd[1:2, :], in_=recip[32:33, :])
            recips[(m, t)] = sd

        def norm_apply(m, t):
            qsl = slice(t * 512, (t + 1) * 512)
            oA, oB = o_tiles.pop((m, t))
            sd = recips.pop((m, t))
            for o_t, row, base in ((oA, 0, 0), (oB, 1, 64)):
                row_ap = sd[row:row + 1, :]
                bsrc = bass.AP(tensor=row_ap.tensor, offset=row_ap.offset,
                               ap=[[0, 64], [1, 512]])
                bcs = small.tile([64, 512], F32, name="bcs", tag="bcs")
                nc.sync.dma_start(out=bcs, in_=bsrc)
                nc.vector.tensor_mul(ot_s[m][base:base + 64, qsl], o_t[0:64, :], bcs)

        fo_tiles = {}

        def outproj_group(t2, mo, engine="vector"):
            if t2 not in fo_tiles:
                fo_tiles[t2] = small.tile([128, 4, 512], F32, name="fo",
                                          tag="fo", bufs=2)
            fo = fo_tiles[t2]
            ps = o_ps.tile([128, 512], F32, name="o", tag="o", bufs=4)
            for m in range(2):
                nc.tensor.matmul(
                    ps,
                    lhsT=wo_s[:, m, mo * 128:(mo + 1) * 128],
                    rhs=ot_s[m][:, t2 * 512:(t2 + 1) * 512],
                    start=(m == 0), stop=(m == 1),
                )
            if engine == "vector":
                nc.vector.tensor_copy(out=fo[:, mo, :], in_=ps)
            else:
                nc.scalar.activation(out=fo[:, mo, :], in_=ps, func=AF.Copy)
            nc.sync.dma_start(
                out=out_d[mo * 128:(mo + 1) * 128, t2 * 512:(t2 + 1) * 512],
                in_=fo[:, mo, :],
            )

        # ---------------- pipelined schedule ----------------
        # 64 global iterations (4 units x 16 kv tiles); scores emitted one
        # iteration ahead so ScalarE's exp stream never waits on PE.
        units = [(0, 0), (1, 0), (0, 1), (1, 1)]
        iters = [(u, i) for u in units for i in range(16)]

        # interleave remaining projections + V tiles + norms + out-proj
        # into the per-iteration PE slack (ACT exp is the steady-state pacer)
        extra = {g: [] for g in range(64)}
        kplan = [(0, 1), (0, 2), (0, 3), (1, 0), (1, 1), (1, 2), (1, 3)]
        for idx, (m_, t_) in enumerate(kplan):
            extra[2 * idx + 1].append(lambda m_=m_, t_=t_: kproj_group(m_, t_))
        extra[0].append(lambda: qproj_group(1, 0))
        for tt in range(16):
            extra[tt].append(lambda tt=tt: vproj_tile(tt))
        post = {
            17: [lambda: norm_recip(0, 0)],
            22: [lambda: norm_apply(0, 0)],
            23: [lambda: qproj_group(0, 1)],
            25: [lambda: qproj_group(1, 1)],
            33: [lambda: norm_recip(1, 0)],
            38: [lambda: norm_apply(1, 0)],
            41: [lambda: outproj_group(0, 0)],
            43: [lambda: outproj_group(0, 1)],
            45: [lambda: outproj_group(0, 2)],
            47: [lambda: outproj_group(0, 3)],
            49: [lambda: norm_recip(0, 1)],
            54: [lambda: norm_apply(0, 1)],
        }

        qproj_group(0, 0)
        kproj_group(0, 0)

        sc_tiles = {}

        def emit_scores(g):
            (m, t), i = iters[g]
            ksl = slice(i * 128, (i + 1) * 128)
            qsl = slice(t * 512, (t + 1) * 512)
            sc = sc_ps.tile([128, 2, 512], F32, name="sc", tag="sc")
            nc.tensor.matmul(
                sc[:, 0, :], lhsT=kt_s[m][0:64, ksl], rhs=qt_s[m][0:64, qsl],
                start=True, stop=True, tile_position=(0, 0),
            )
            nc.tensor.matmul(
                sc[:, 1, :], lhsT=kt_s[m][64:128, ksl], rhs=qt_s[m][64:128, qsl],
                start=True, stop=True, tile_position=(64, 0),
            )
            sc_tiles[g] = sc

        emit_scores(0)
        for g in range(64):
            (m, t), i = iters[g]
            if g + 1 < 64:
                emit_scores(g + 1)
            sc = sc_tiles.pop(g)
            e = esb.tile([128, 2, 512], BF16, name="e", tag="e")
            nc.scalar.activation(out=e[:], in_=sc[:], func=AF.Exp, scale=0.125)
            for fn in extra.get(g, ()):
                fn()
            if i == 0:
                oA = o_ps.tile([65, 512], F32, name="o", tag="o", bufs=4)
                oB = o_ps.tile([65, 512], F32, name="o", tag="o", bufs=4)
                o_tiles[(m, t)] = (oA, oB)
            oA, oB = o_tiles[(m, t)]
            jA, jB = 2 * m, 2 * m + 1
            nc.tensor.matmul(
                oA, lhsT=v_s[:, i, 65 * jA:65 * jA + 65], rhs=e[:, 0, :],
                start=(i == 0), stop=(i == 15),
            )
            nc.tensor.matmul(
                oB, lhsT=v_s[:, i, 65 * jB:65 * jB + 65], rhs=e[:, 1, :],
                start=(i == 0), stop=(i == 15),
            )
            for fn in post.get(g, ()):
                fn()

        norm_recip(1, 1)
        norm_apply(1, 1)
        for mo in range(4):
            outproj_group(1, mo, engine="scalar")

    _split_multi_waits(nc)
    return nc


_PROGRAM = None


def _get_program() -> bass.Bass:
    global _PROGRAM
    if _PROGRAM is None:
        _PROGRAM = _build_program()
    return _PROGRAM


def _prep_core_inputs(c, q, kv, Wqkv, bqkv, Wout):
    b, g = c // 2, c % 2
    cs = slice(256 * g, 256 * g + 256)
    wv_base = Wqkv[1024 + 256 * g:1024 + 256 * g + 256, :].T  # [512, 256]
    wv = np.zeros((513, 260), np.float32)
    for j in range(4):
        wv[0:512, 65 * j:65 * j + 64] = wv_base[:, 64 * j:64 * j + 64]
        wv[512, 65 * j + 64] = 1.0
    return {
        "qx": np.ascontiguousarray(q[b].reshape(512, 1024)).astype(NP_BF16),
        "kvx": np.ascontiguousarray(kv[b].reshape(512, 2048)).astype(NP_BF16),
        "wq": np.ascontiguousarray(Wqkv[cs, :].T).astype(NP_BF16),
        "wk": np.ascontiguousarray(Wqkv[512 + 256 * g:512 + 256 * g + 256, :].T).astype(NP_BF16),
        "wv": wv.astype(NP_BF16),
        "wo": np.ascontiguousarray(Wout[:, cs].T).astype(NP_BF16),
        "bq": np.ascontiguousarray(bqkv[cs].reshape(2, 128).T).astype(np.float32),
        "bk": np.ascontiguousarray(bqkv[512 + 256 * g:512 + 256 * g + 256].reshape(2, 128).T).astype(np.float32),
    }


def kernel(q, kv, Wqkv, bqkv, Wout, bout):
    q = np.asarray(q, np.float32)
    kv = np.asarray(kv, np.float32)
    Wqkv = np.asarray(Wqkv, np.float32)
    bqkv = np.asarray(bqkv, np.float32)
    Wout = np.asarray(Wout, np.float32)
    bout = np.asarray(bout, np.float32)

    nc = _get_program()
    in_maps = [_prep_core_inputs(c, q, kv, Wqkv, bqkv, Wout) for c in range(8)]
    res = run_bass_kernel_spmd(nc, in_maps, list(range(8))).results

    # V-bias folds through softmax (rows sum to 1): bout' = bout + Wout @ bv
    bout_adj = bout + Wout @ bqkv[1024:1536]
    out = np.empty((4, 512, 32, 32), np.float32)
    for b in range(4):
        o = res[2 * b]["out"] + res[2 * b + 1]["out"] + bout_adj[:, None]
        out[b] = o.reshape(512, 32, 32)
    return out

